# revision 1
# baseline (speedup 1.0000x reference)
"""DeepQI dense MLP on 8 Trainium2 NeuronCores.

Strategy (data-parallel, feature-major):
  - Shard batch B=16384 across 8 cores (2048 rows each); weights replicated.
  - On-chip layout is feature-major: activations live as [128, KT, B_c]
    SBUF tiles (feature on partitions, batch on free dim), so every layer is
    a matmul with K on partitions, zero transposes between layers, and
    BatchNorm stats are free-dim reductions.
  - Pairwise features qi.T = (x_i * x_j)/255 are built on-chip: A = PII @ xT,
    Bm = PJJ @ xT via 0/1 selection matmuls (K=64), then one DVE
    scalar_tensor_tensor pass (A * (1/255)) * Bm -> bf16.
  - BatchNorm is sync-BN: per-core bn_stats/bn_aggr (DVE), tiny [128, MT, 2]
    AllReduce of (mean/8, E[x^2]/8) per layer, then scale/shift+relu fused in
    one ACT pass per m-tile.  Linear biases b0/b1/b2 cancel in BN and are
    dropped entirely.
  - All matmuls in bf16 (fp32 PSUM accumulate): measured end-to-end rel err
    ~7e-3 vs the fp32 reference.
"""

import numpy as np
import ml_dtypes

import concourse.bass as bass
import concourse.mybir as mybir
import concourse.tile as tile
from concourse import bacc
from concourse.bass import ts
from concourse.bass_utils import run_bass_kernel_spmd

N_CORES = 8
P = 128
B = 16384
BC = B // N_CORES  # 2048 batch rows per core
F = 256            # xv feature dim
FIELD = 64
NPAIR = 2016
NPAIR_PAD = 2048   # pad pairs to 16 full k-tiles
D1, D2, D3 = 2048, 2048, 1024
NCHUNK = 512       # matmul moving free dim (one PSUM bank)
EPS = 1e-5

BF16 = mybir.dt.bfloat16
F32 = mybir.dt.float32
AF = mybir.ActivationFunctionType
ALU = mybir.AluOpType

_II, _JJ = np.triu_indices(FIELD, k=1)

KT0 = (F + NPAIR_PAD) // P          # 18
MT1, MT2, MT3 = D1 // P, D2 // P, D3 // P   # 16, 16, 8
KT3 = D3 // P                        # 8


def build_nc(bc=BC, mm_reps=1):
    # mm_reps > 1 redundantly recomputes every m-strip (identical results,
    # ~mm_reps x the matmul device time) — used only to amplify device time
    # above the axon relay's wall-clock noise floor for timing.
    assert bc % NCHUNK == 0
    nch = bc // NCHUNK

    nc = bacc.Bacc("TRN2", target_bir_lowering=False, debug=False,
                   num_devices=N_CORES)

    xt_d = nc.dram_tensor("xt", [2, P, bc], BF16, kind="ExternalInput")
    pii_d = nc.dram_tensor("pii", [FIELD, NPAIR_PAD], BF16, kind="ExternalInput")
    pjj_d = nc.dram_tensor("pjj", [FIELD, NPAIR_PAD], BF16, kind="ExternalInput")
    w0_d = nc.dram_tensor("w0t", [KT0, P, D1], BF16, kind="ExternalInput")
    w1_d = nc.dram_tensor("w1t", [MT1, P, D2], BF16, kind="ExternalInput")
    w2_d = nc.dram_tensor("w2t", [MT2, P, D3], BF16, kind="ExternalInput")
    w3_d = nc.dram_tensor("w3t", [P, KT3], BF16, kind="ExternalInput")
    gbe0_d = nc.dram_tensor("gbe0", [P, MT1, 2], F32, kind="ExternalInput")
    gbe1_d = nc.dram_tensor("gbe1", [P, MT2, 2], F32, kind="ExternalInput")
    gbe2_d = nc.dram_tensor("gbe2", [P, MT3, 2], F32, kind="ExternalInput")
    bout_d = nc.dram_tensor("bout3", [1, 1], F32, kind="ExternalInput")
    out_d = nc.dram_tensor("out", [1, bc], F32, kind="ExternalOutput")

    with tile.TileContext(nc) as tc:
        with (
            tc.tile_pool(name="sb", bufs=1) as sb,
            tc.tile_pool(name="wpool", bufs=4) as wpool,
            tc.tile_pool(name="pspool", bufs=8, space="PSUM") as pspool,
            tc.tile_pool(name="small", bufs=1) as small,
            tc.tile_pool(name="dram", bufs=1, space="DRAM") as dram,
        ):
            # ---- persistent activations ----
            X = sb.tile([P, 2, bc], BF16, tag="X", name="X")
            # "big" tag cycles: Q(qi) -> A1(act1) -> A2(act2) -> A3(act3)
            Q = sb.tile([P, 16, bc], BF16, tag="big", name="Q")
            # "h" tag cycles: H0 -> H1 -> H2 (pre-BN outputs)
            H0 = sb.tile([P, 16, bc], BF16, tag="h", name="H0")

            nc.sync.dma_start(X[:], xt_d.ap().rearrange("t p b -> p t b"))
            # PII on partitions 0-63, PJJ on 64-127: the two K=64 selection
            # matmuls then run CONCURRENTLY as 64x128 row tiles (T0/T8).
            pp_sb = small.tile([P, NPAIR_PAD], BF16, tag="pp_sb", name="pp_sb")
            nc.sync.dma_start(pp_sb[0:FIELD, :], pii_d.ap())
            nc.sync.dma_start(pp_sb[FIELD:P, :], pjj_d.ap())
            # duplicate of x features 0-63 on partitions 64-127 (T8's rhs
            # must stream from SBUF partitions 64-127)
            xx_sb = small.tile([P, bc], BF16, tag="xx_sb", name="xx_sb")
            nc.sync.dma_start(xx_sb[FIELD:P, :], xt_d.ap()[0, 0:FIELD, :])

            gbe_sb = {}
            for li, (gbe_d, mt_n) in enumerate(
                    [(gbe0_d, MT1), (gbe1_d, MT2), (gbe2_d, MT3)]):
                g = small.tile([P, mt_n, 2], F32, tag=f"gbe{li}", name=f"gbe{li}")
                nc.sync.dma_start(g[:], gbe_d.ap())
                gbe_sb[li] = g
            w3_sb = small.tile([P, KT3], BF16, tag="w3_sb", name="w3_sb")
            nc.sync.dma_start(w3_sb[:], w3_d.ap())
            bout_sb = small.tile([1, 1], F32, tag="bout_sb", name="bout_sb")
            nc.sync.dma_start(bout_sb[:], bout_d.ap())
            eps_sb = small.tile([P, 1], F32, tag="eps_sb", name="eps_sb")
            nc.vector.memset(eps_sb[:], EPS)

            # ---- build qi tiles: Q[:, q, c] = (PII@x * (1/255)) * (PJJ@x) ----
            # DVE two-tensor ops allow at most one PSUM operand, so the
            # PJJ product bounces through SBUF (values are bf16-exact).
            for q in range(16):
                for c in range(nch):
                    a_ps = pspool.tile([P, NCHUNK], F32, tag="ps", name="aps")
                    b_ps = pspool.tile([P, NCHUNK], F32, tag="ps", name="bps")
                    nc.tensor.matmul(a_ps[:], pp_sb[0:FIELD, ts(q, P)],
                                     X[0:FIELD, 0, ts(c, NCHUNK)],
                                     start=True, stop=True,
                                     tile_position=(0, 0))
                    nc.tensor.matmul(b_ps[:], pp_sb[FIELD:P, ts(q, P)],
                                     xx_sb[FIELD:P, ts(c, NCHUNK)],
                                     start=True, stop=True,
                                     tile_position=(64, 0))
                    b_sb = wpool.tile([P, NCHUNK], BF16, tag="qtmp",
                                      name="b_sb", bufs=3)
                    nc.scalar.copy(b_sb[:], b_ps[:])
                    nc.vector.scalar_tensor_tensor(
                        out=Q[:, q, ts(c, NCHUNK)],
                        in0=a_ps[:], scalar=1.0 / 255.0, in1=b_sb[:],
                        op0=ALU.mult, op1=ALU.mult)

            def l0_rhs(kt, c):
                if kt < 2:
                    return X[:, kt, ts(c, NCHUNK)]
                return Q[:, kt - 2, ts(c, NCHUNK)]

            def dense_bn_relu(li, kt_n, mt_n, w_d, rhs_fn, h_buf):
                """h = act @ W.T ; sync-BN ; relu(s*h + t) IN-PLACE on h_buf.

                Sync-BN is split into two per-half AllReduces so the first
                one hides under the second half's matmuls, and the relu of
                each half runs on ACT (even m-tiles) + DVE (odd) in parallel,
                overlapping the tail AR / next layer's first k-loops.
                """
                stats6 = small.tile([P, mt_n, nch, 6], F32, tag=f"st6_{li}", name=f"st6_{li}")
                mv = small.tile([P, mt_n, 2], F32, tag=f"mv_{li}", name=f"mv_{li}")
                # asymmetric split: the big first AR hides under the last
                # quarter's matmuls; the small tail AR has more relu'd
                # k-tiles of PE cover in the next layer.
                s_pt = (3 * mt_n) // 4
                for half, (h0, h1) in enumerate([(0, s_pt), (s_pt, mt_n)]):
                    HALF = h1 - h0
                    for mt in range(h0, h1):
                        for _rep in range(mm_reps):
                            w_sb = wpool.tile([P, kt_n, P], BF16, tag="w",
                                              name="w_sb")
                            nc.sync.dma_start(
                                w_sb[:],
                                w_d.ap()[:, :, ts(mt, P)]
                                .rearrange("k p m -> p k m"))
                            ps = [pspool.tile([P, NCHUNK], F32, tag="ps",
                                              name="mps")
                                  for _ in range(nch)]
                            for kt in range(kt_n):
                                for c in range(nch):
                                    nc.tensor.matmul(ps[c][:], w_sb[:, kt, :],
                                                     rhs_fn(kt, c),
                                                     start=(kt == 0),
                                                     stop=(kt == kt_n - 1))
                            for c in range(nch):
                                nc.scalar.copy(h_buf[:, mt, ts(c, NCHUNK)],
                                               ps[c][:])
                        for c in range(nch):
                            nc.vector.bn_stats(stats6[:, mt, c, :],
                                               h_buf[:, mt, ts(c, NCHUNK)])
                        nc.vector.bn_aggr(mv[:, mt, :], stats6[:, mt, :, :])

                    # pack (mean/8, E[x^2]/8) for this half and AllReduce
                    hs = f"{li}_{half}"
                    mvh = mv[:, h0:h0 + HALF, :]
                    arp = small.tile([P, HALF, 2], F32, tag=f"arp_{hs}", name=f"arp_{hs}")
                    nc.vector.tensor_scalar_mul(arp[:, :, 0], mvh[:, :, 0],
                                                1.0 / N_CORES)
                    e2 = small.tile([P, HALF], F32, tag=f"e2_{hs}", name=f"e2_{hs}")
                    nc.vector.tensor_mul(e2[:], mvh[:, :, 0], mvh[:, :, 0])
                    nc.vector.tensor_add(e2[:], e2[:], mvh[:, :, 1])
                    nc.vector.tensor_scalar_mul(arp[:, :, 1], e2[:], 1.0 / N_CORES)
                    arin = dram.tile([P, HALF, 2], F32, tag=f"arin_{hs}", name=f"arin_{hs}")
                    arout = dram.tile([P, HALF, 2], F32, tag=f"arout_{hs}", name=f"arout_{hs}")
                    nc.sync.dma_start(arin[:], arp[:])
                    nc.gpsimd.collective_compute(
                        "AllReduce", ALU.add,
                        replica_groups=[list(range(N_CORES))],
                        ins=[arin.opt()], outs=[arout.opt()])
                    gl = small.tile([P, HALF, 2], F32, tag=f"gl_{hs}", name=f"gl_{hs}")
                    nc.sync.dma_start(gl[:], arout[:])

                    # s = g / sqrt(var+eps) ; t = be - mean*s
                    var = small.tile([P, HALF], F32, tag=f"var_{hs}", name=f"var_{hs}")
                    nc.vector.tensor_mul(var[:], gl[:, :, 0], gl[:, :, 0])
                    nc.vector.tensor_sub(var[:], gl[:, :, 1], var[:])
                    sd = small.tile([P, HALF], F32, tag=f"sd_{hs}", name=f"sd_{hs}")
                    nc.scalar.activation(sd[:], var[:], AF.Sqrt, bias=eps_sb[:])
                    s_t = small.tile([P, HALF], F32, tag=f"s_{hs}", name=f"s_{hs}")
                    nc.vector.reciprocal(s_t[:], sd[:])
                    nc.vector.tensor_mul(s_t[:], s_t[:],
                                         gbe_sb[li][:, h0:h0 + HALF, 0])
                    t_t = small.tile([P, HALF], F32, tag=f"t_{hs}", name=f"t_{hs}")
                    nc.vector.tensor_mul(t_t[:], gl[:, :, 0], s_t[:])
                    nc.vector.tensor_sub(t_t[:], gbe_sb[li][:, h0:h0 + HALF, 1],
                                         t_t[:])

                    for j in range(HALF):
                        mt = h0 + j
                        s_ap = s_t[:, j:j + 1]
                        t_ap = t_t[:, j:j + 1]
                        if j % 2 == 0:
                            nc.scalar.activation(h_buf[:, mt, :],
                                                 h_buf[:, mt, :], AF.Relu,
                                                 bias=t_ap, scale=s_ap)
                        else:
                            nc.vector.tensor_scalar(
                                out=h_buf[:, mt, :], in0=h_buf[:, mt, :],
                                scalar1=s_ap, scalar2=t_ap,
                                op0=ALU.mult, op1=ALU.add)
                            nc.vector.tensor_scalar_max(
                                h_buf[:, mt, :], h_buf[:, mt, :], 0.0)

            # ---- layer 0: reads X+Q, h0 -> H0 (relu'd in place) ----
            dense_bn_relu(0, KT0, MT1, w0_d, l0_rhs, H0)

            # ---- layer 1: reads H0, h1 -> H1 (reuses Q's memory) ----
            H1 = sb.tile([P, 16, bc], BF16, tag="big", name="H1")
            dense_bn_relu(1, MT1, MT2, w1_d,
                          lambda kt, c: H0[:, kt, ts(c, NCHUNK)], H1)

            # ---- layer 2: reads H1, h2 -> H2 (reuses H0's memory) ----
            H2 = sb.tile([P, 16, bc], BF16, tag="h", name="H2")
            dense_bn_relu(2, MT2, MT3, w2_d,
                          lambda kt, c: H1[:, kt, ts(c, NCHUNK)], H2)

            # ---- output layer: out[1, bc] = act3 @ Wout.T + bout ----
            out_sb = small.tile([1, bc], F32, tag="out_sb", name="out_sb")
            for c in range(nch):
                ps3 = pspool.tile([P, NCHUNK], F32, tag="ps", name="ps3")
                for kt in range(KT3):
                    nc.tensor.matmul(ps3[0:1, :], w3_sb[:, kt:kt + 1],
                                     H2[:, kt, ts(c, NCHUNK)],
                                     start=(kt == 0), stop=(kt == KT3 - 1))
                nc.scalar.activation(out_sb[:, ts(c, NCHUNK)], ps3[0:1, :],
                                     AF.Identity, bias=bout_sb[:])
            nc.sync.dma_start(out_d.ap(), out_sb[:])

    nc.compile()
    return nc


# ---------------------------------------------------------------------------
# host-side prep + execution
# ---------------------------------------------------------------------------

_NC_CACHE = {}


def _get_nc(bc=BC, mm_reps=1):
    key = (bc, mm_reps)
    if key not in _NC_CACHE:
        _NC_CACHE[key] = build_nc(bc, mm_reps)
    return _NC_CACHE[key]


def _bf16(a):
    return np.ascontiguousarray(a).astype(ml_dtypes.bfloat16)


def prep_in_maps(inputs, bc=BC, n_cores=N_CORES):
    xv = np.asarray(inputs["xv"], dtype=np.float32)
    W0 = np.asarray(inputs["W0"], dtype=np.float32)
    W1 = np.asarray(inputs["W1"], dtype=np.float32)
    W2 = np.asarray(inputs["W2"], dtype=np.float32)
    Wout = np.asarray(inputs["Wout"], dtype=np.float32)
    bout = np.asarray(inputs["bout"], dtype=np.float32)

    pii = np.zeros((FIELD, NPAIR_PAD), np.float32)
    pjj = np.zeros((FIELD, NPAIR_PAD), np.float32)
    pii[_II, np.arange(NPAIR)] = 1.0
    pjj[_JJ, np.arange(NPAIR)] = 1.0

    w0t = np.vstack([W0.T, np.zeros((KT0 * P - (F + NPAIR), D1), np.float32)])
    shared = {
        "pii": _bf16(pii),
        "pjj": _bf16(pjj),
        "w0t": _bf16(w0t.reshape(KT0, P, D1)),
        "w1t": _bf16(W1.T.reshape(MT1, P, D2)),
        "w2t": _bf16(W2.T.reshape(MT2, P, D3)),
        "w3t": _bf16(Wout.reshape(KT3, P).T),
        "bout3": bout.reshape(1, 1).astype(np.float32),
    }
    for li, (g, be, mt_n) in enumerate([
            (inputs["g0"], inputs["be0"], MT1),
            (inputs["g1"], inputs["be1"], MT2),
            (inputs["g2"], inputs["be2"], MT3)]):
        g = np.asarray(g, np.float32).reshape(mt_n, P).T
        be = np.asarray(be, np.float32).reshape(mt_n, P).T
        shared[f"gbe{li}"] = np.ascontiguousarray(
            np.stack([g, be], axis=-1), dtype=np.float32)

    in_maps = []
    for c in range(n_cores):
        xs = xv[c * bc:(c + 1) * bc, :]                     # [bc, F]
        xt = _bf16(xs.T.reshape(2, P, bc))
        m = dict(shared)
        m["xt"] = xt
        in_maps.append(m)
    return in_maps


def kernel(**inputs):
    nc = _get_nc(BC)
    in_maps = prep_in_maps(inputs)
    res = run_bass_kernel_spmd(nc, in_maps, core_ids=list(range(N_CORES)))
    out = np.concatenate(
        [res.results[c]["out"].reshape(BC) for c in range(N_CORES)])
    return out.reshape(B, 1).astype(np.float32)



# revision 11
# speedup vs baseline: 2.3703x; 2.3703x over previous
"""DeepQI dense MLP on 8 Trainium2 NeuronCores.

Strategy (data-parallel, feature-major):
  - Shard batch B=16384 across 8 cores (2048 rows each); weights replicated.
  - Feature-major on-chip layout: activations are [128, KT, B_c] SBUF tiles
    (feature on partitions, batch on free dim) so every layer is a matmul
    with K on partitions and BatchNorm stats are free-dim reductions.
  - Pairwise features qi.T = x_i * x_j are built on-chip (selection matmuls
    PII/PJJ run concurrently as 64-row PE tiles), stored as fp8e4m3; the
    reference's /255 is folded into W0's qi columns, which are stored as
    fp8e5m2 (/255 keeps them in e5m2's normal range).  The 16 qi k-tiles of
    layer 0 then run as 8 fp8 DoubleRow matmuls (2x PE throughput).  The qi
    path carries ~0.01% of h0's variance (the /255 damping), so fp8 error
    is negligible end-to-end.
  - BatchNorm is sync-BN via two small AllReduces per layer, split
    asymmetrically (most strips in AR1, 2-3 tail strips in AR2) so AR1 hides
    under this layer's tail matmuls and AR2's chain hides under the NEXT
    layer's matmuls: the next layer starts on the k-tiles whose relu is
    already done (2 strips interleaved in PSUM = ~24 us of cover), deferring
    the late k-tiles to the end of each accumulation.  bn_stats run directly
    on PSUM so they never wait for the ACT copy.  Linear biases cancel in BN
    and are dropped.
  - Matmuls in bf16 (fp32 PSUM accumulate) except the qi DoubleRow part.
"""

import numpy as np
import ml_dtypes

import concourse.bass as bass
import concourse.mybir as mybir
import concourse.tile as tile
from concourse import bacc
from concourse.bass import ts
from concourse.bass_utils import run_bass_kernel_spmd

N_CORES = 8
P = 128
B = 16384
BC = B // N_CORES  # 2048 batch rows per core
F = 256            # xv feature dim
FIELD = 64
NPAIR = 2016
NPAIR_PAD = 2048   # pad pairs to 16 full k-tiles
D1, D2, D3 = 2048, 2048, 1024
NCHUNK = 512       # matmul moving free dim (one PSUM bank)
EPS = 1e-5

BF16 = mybir.dt.bfloat16
F32 = mybir.dt.float32
F8E4 = mybir.dt.float8e4   # e4m3
F8E5 = mybir.dt.float8e5   # e5m2
AF = mybir.ActivationFunctionType
ALU = mybir.AluOpType
DR = mybir.MatmulPerfMode.DoubleRow

_II, _JJ = np.triu_indices(FIELD, k=1)

KT0 = (F + NPAIR_PAD) // P          # 18 (2 bf16 + 16 fp8)
QKT = NPAIR_PAD // P                 # 16 qi k-tiles -> 8 DoubleRow pairs
MT1, MT2, MT3 = D1 // P, D2 // P, D3 // P   # 16, 16, 8
KT3 = D3 // P                        # 8

# (name, shape, mybir dtype) of every kernel input -- used by test.py's
# null kernel to mirror the I/O signature.
INPUT_SPECS = [
    ("xt", [2, P, BC], BF16),
    ("pii", [FIELD, NPAIR_PAD], BF16),
    ("pjj", [FIELD, NPAIR_PAD], BF16),
    ("w0x", [MT1, P, 2, P], BF16),
    ("w0q", [MT1, P, QKT, P], F8E5),
    ("w1t", [MT2, P, MT1, P], BF16),
    ("w2t", [MT3, P, MT2, P], BF16),
    ("w3t", [P, KT3], BF16),
    ("gbe0", [P, MT1, 2], F32),
    ("gbe1", [P, MT2, 2], F32),
    ("gbe2", [P, MT3, 2], F32),
    ("bout3", [1, 1], F32),
]


def build_nc(bc=BC):
    assert bc % NCHUNK == 0
    nch = bc // NCHUNK

    nc = bacc.Bacc("TRN2", target_bir_lowering=False, debug=False,
                   num_devices=N_CORES)

    d = {name: nc.dram_tensor(name, shape, dt, kind="ExternalInput")
         for name, shape, dt in INPUT_SPECS}
    out_d = nc.dram_tensor("out", [1, bc], F32, kind="ExternalOutput")

    with tile.TileContext(nc) as tc:
        with (
            tc.tile_pool(name="sb", bufs=1) as sb,
            tc.tile_pool(name="wpool", bufs=4) as wpool,
            tc.tile_pool(name="pspool", bufs=8, space="PSUM") as pspool,
            tc.tile_pool(name="small", bufs=1) as small,
            tc.tile_pool(name="dram", bufs=1, space="DRAM") as dram,
        ):
            # ---- persistent activations ----
            X = sb.tile([P, 2, bc], BF16, tag="X", name="X")
            # Q shares H1's 64KB region (Q is read only during L0; H1 is
            # written from L1 on).  H2 likewise shares H0's region.
            Q = sb.tile([P, QKT, bc], F8E4, tag="h1q", name="Q")
            H0 = sb.tile([P, MT1, bc], BF16, tag="h0q", name="H0")

            nc.sync.dma_start(X[:], d["xt"].ap().rearrange("t p b -> p t b"))
            # PII on partitions 0-63, PJJ on 64-127: the two K=64 selection
            # matmuls then run CONCURRENTLY as 64x128 row tiles.
            pp_sb = small.tile([P, NPAIR_PAD], BF16, tag="pp_sb", name="pp_sb")
            nc.sync.dma_start(pp_sb[0:FIELD, :], d["pii"].ap())
            nc.sync.dma_start(pp_sb[FIELD:P, :], d["pjj"].ap())
            # duplicate of x features 0-63 on partitions 64-127 (the second
            # matmul's rhs must stream from SBUF partitions 64-127)
            xx_sb = small.tile([P, bc], BF16, tag="xx_sb", name="xx_sb")
            nc.sync.dma_start(xx_sb[FIELD:P, :], d["xt"].ap()[0, 0:FIELD, :])

            gbe_sb = {}
            for li, mt_n in [(0, MT1), (1, MT2), (2, MT3)]:
                g = small.tile([P, mt_n, 2], F32, tag=f"gbe{li}", name=f"gbe{li}")
                nc.sync.dma_start(g[:], d[f"gbe{li}"].ap())
                gbe_sb[li] = g
            w3_sb = small.tile([P, KT3], BF16, tag="w3_sb", name="w3_sb")
            nc.sync.dma_start(w3_sb[:], d["w3t"].ap())
            bout_sb = small.tile([1, 1], F32, tag="bout_sb", name="bout_sb")
            nc.sync.dma_start(bout_sb[:], d["bout3"].ap())
            eps_sb = small.tile([P, 1], F32, tag="eps_sb", name="eps_sb")
            nc.vector.memset(eps_sb[:], EPS)

            # ---- build qi tiles: Q[:, q, c] = (PII@x) * (PJJ@x), fp8e4 ----
            for q in range(QKT):
                for c in range(nch):
                    a_ps = pspool.tile([P, NCHUNK], F32, tag="ps", name="aps")
                    b_ps = pspool.tile([P, NCHUNK], F32, tag="ps", name="bps")
                    nc.tensor.matmul(a_ps[:], pp_sb[0:FIELD, ts(q, P)],
                                     X[0:FIELD, 0, ts(c, NCHUNK)],
                                     start=True, stop=True,
                                     tile_position=(0, 0))
                    nc.tensor.matmul(b_ps[:], pp_sb[FIELD:P, ts(q, P)],
                                     xx_sb[FIELD:P, ts(c, NCHUNK)],
                                     start=True, stop=True,
                                     tile_position=(64, 0))
                    b_sb = wpool.tile([P, NCHUNK], BF16, tag="qtmp",
                                      name="b_sb", bufs=3)
                    nc.scalar.copy(b_sb[:], b_ps[:])
                    nc.vector.scalar_tensor_tensor(
                        out=Q[:, q, ts(c, NCHUNK)],
                        in0=a_ps[:], scalar=1.0, in1=b_sb[:],
                        op0=ALU.mult, op1=ALU.mult)

            # ------------------------------------------------------------------
            # layer machinery
            # ------------------------------------------------------------------
            def issue_ar(li, tag, mv, h0, h1):
                """Pack (mean/8, E[x^2]/8) for strips [h0,h1) and AllReduce."""
                HALF = h1 - h0
                hs = f"{li}_{tag}"
                mvh = mv[:, h0:h1, :]
                arp = small.tile([P, HALF, 2], F32, tag=f"arp_{hs}", name=f"arp_{hs}")
                nc.vector.tensor_scalar_mul(arp[:, :, 0], mvh[:, :, 0],
                                            1.0 / N_CORES)
                e2 = small.tile([P, HALF], F32, tag=f"e2_{hs}", name=f"e2_{hs}")
                nc.vector.tensor_mul(e2[:], mvh[:, :, 0], mvh[:, :, 0])
                nc.vector.tensor_add(e2[:], e2[:], mvh[:, :, 1])
                nc.vector.tensor_scalar_mul(arp[:, :, 1], e2[:], 1.0 / N_CORES)
                arin = dram.tile([P, HALF, 2], F32, tag=f"arin_{hs}", name=f"arin_{hs}")
                arout = dram.tile([P, HALF, 2], F32, tag=f"arout_{hs}", name=f"arout_{hs}")
                nc.sync.dma_start(arin[:], arp[:])
                nc.gpsimd.collective_compute(
                    "AllReduce", ALU.add,
                    replica_groups=[list(range(N_CORES))],
                    ins=[arin.opt()], outs=[arout.opt()])
                gl = small.tile([P, HALF, 2], F32, tag=f"gl_{hs}", name=f"gl_{hs}")
                nc.sync.dma_start(gl[:], arout[:])
                return gl

            def st_from_gl(li, tag, gl, h0, h1):
                """s = g / sqrt(var+eps); t = be - mean*s for strips [h0,h1)."""
                HALF = h1 - h0
                hs = f"{li}_{tag}"
                var = small.tile([P, HALF], F32, tag=f"var_{hs}", name=f"var_{hs}")
                nc.vector.tensor_mul(var[:], gl[:, :, 0], gl[:, :, 0])
                nc.vector.tensor_sub(var[:], gl[:, :, 1], var[:])
                sd = small.tile([P, HALF], F32, tag=f"sd_{hs}", name=f"sd_{hs}")
                nc.scalar.activation(sd[:], var[:], AF.Sqrt, bias=eps_sb[:])
                s_t = small.tile([P, HALF], F32, tag=f"s_{hs}", name=f"s_{hs}")
                nc.vector.reciprocal(s_t[:], sd[:])
                nc.vector.tensor_mul(s_t[:], s_t[:], gbe_sb[li][:, h0:h1, 0])
                t_t = small.tile([P, HALF], F32, tag=f"t_{hs}", name=f"t_{hs}")
                nc.vector.tensor_mul(t_t[:], gl[:, :, 0], s_t[:])
                nc.vector.tensor_sub(t_t[:], gbe_sb[li][:, h0:h1, 1], t_t[:])
                return s_t, t_t

            def relu_strip(h_buf, mt, s_t, t_t, j, on_act):
                s_ap = s_t[:, j:j + 1]
                t_ap = t_t[:, j:j + 1]
                if on_act:
                    nc.scalar.activation(h_buf[:, mt, :], h_buf[:, mt, :],
                                         AF.Relu, bias=t_ap, scale=s_ap)
                else:
                    nc.vector.tensor_scalar(
                        out=h_buf[:, mt, :], in0=h_buf[:, mt, :],
                        scalar1=s_ap, scalar2=t_ap,
                        op0=ALU.mult, op1=ALU.add)
                    nc.vector.tensor_scalar_max(
                        h_buf[:, mt, :], h_buf[:, mt, :], 0.0)

            def load_w(li, w_d, kt_n, mt, wdt, bufs=3):
                w_sb = wpool.tile([P, kt_n, P], wdt, tag=f"w_{kt_n}_{wdt}",
                                  name=f"w{li}_sb", bufs=bufs)
                nc.sync.dma_start(w_sb[:], w_d.ap()[mt])
                return w_sb

            def copy_out(h_buf, mt, ps, on_act):
                # GPSIMD cannot read PSUM, so these all go on ACT
                for c in range(nch):
                    nc.scalar.copy(h_buf[:, mt, ts(c, NCHUNK)], ps[c][:])

            def stats_psum(li, mt, ps, stats6, mv):
                for c in range(nch):
                    nc.vector.bn_stats(stats6[:, mt, c, :], ps[c][:])
                nc.vector.bn_aggr(mv[:, mt, :], stats6[:, mt, :, :])

            def stats_sbuf(li, h_buf, mt, stats6, mv):
                for c in range(nch):
                    nc.vector.bn_stats(stats6[:, mt, c, :],
                                       h_buf[:, mt, ts(c, NCHUNK)])
                nc.vector.bn_aggr(mv[:, mt, :], stats6[:, mt, :, :])

            N_SPLIT = 4   # strips per layer that drain ready-k partials so
                          # the prev layer's tail AR hides under deep PE work

            def emit_layer(li, mt_n, h_buf, mm_ready, mm_tail, has_late,
                           prev_finisher, split, tail_s):
                stats6 = small.tile([P, 16, nch, 6], F32,
                                    tag="st6", name=f"st6_{li}")
                mv = small.tile([P, mt_n, 2], F32, tag=f"mv_{li}",
                                name=f"mv_{li}")

                def alloc_ps():
                    return [pspool.tile([P, NCHUNK], F32, tag="ps", name="mps")
                            for _ in range(nch)]

                if has_late:
                    # phase 1: ready-k partial sums of the first N_SPLIT
                    # strips, drained to SBUF (frees PSUM -> deep cover)
                    for s in range(N_SPLIT):
                        ps = alloc_ps()
                        mm_ready(s, ps, partial=True)
                        copy_out(h_buf, s, ps, on_act=(s % 2 == 0))
                    prev_finisher()
                    # phase 2: late-k contributions, added in place
                    for s in range(N_SPLIT):
                        ps = alloc_ps()
                        mm_tail(s, ps, fresh=True)
                        for c in range(nch):
                            nc.vector.tensor_add(
                                h_buf[:, s, ts(c, NCHUNK)],
                                h_buf[:, s, ts(c, NCHUNK)], ps[c][:])
                        stats_sbuf(li, h_buf, s, stats6, mv)
                    start_s = N_SPLIT
                else:
                    if prev_finisher is not None:
                        prev_finisher()
                    start_s = 0

                gl1 = None
                for mt in range(start_s, mt_n):
                    ps = alloc_ps()
                    mm_ready(mt, ps, partial=False)
                    mm_tail(mt, ps, fresh=False)
                    stats_psum(li, mt, ps, stats6, mv)
                    copy_out(h_buf, mt, ps, on_act=(mt % 2 == 0))
                    if mt == split - 1:
                        gl1 = issue_ar(li, "a", mv, 0, split)

                gl2 = issue_ar(li, "b", mv, split, mt_n)
                st1 = st_from_gl(li, "a", gl1, 0, split)
                for j in range(split):
                    relu_strip(h_buf, j, st1[0], st1[1], j, on_act=(j % 2 == 0))

                def finisher():
                    st2 = st_from_gl(li, "b", gl2, split, mt_n)
                    for j in range(mt_n - split):
                        relu_strip(h_buf, split + j, st2[0], st2[1], j,
                                   on_act=False)
                return finisher

            # ---- layer 0: X (2 bf16 kts) + Q (8 fp8 DoubleRow pairs) ----
            def l0_mm(mt, ps, partial=False):
                wx = load_w(0, d["w0x"], 2, mt, BF16)
                wq = load_w("0q", d["w0q"], QKT, mt, F8E5)
                for kt in range(2):
                    for c in range(nch):
                        nc.tensor.matmul(ps[c][:], wx[:, kt, :],
                                         X[:, kt, ts(c, NCHUNK)],
                                         start=(kt == 0), stop=False)
                for dkt in range(QKT // 2):
                    for c in range(nch):
                        nc.tensor.matmul(
                            ps[c][:], wq[:, 2 * dkt:2 * dkt + 2, :],
                            Q[:, 2 * dkt:2 * dkt + 2, ts(c, NCHUNK)],
                            start=False, stop=(dkt == QKT // 2 - 1),
                            perf_mode=DR)

            fin0 = emit_layer(0, MT1, H0, l0_mm,
                              lambda mt, ps, fresh=False: None,
                              False, None, split=10, tail_s=6)

            # ---- layers 1, 2: dense bf16 over previous h ----
            def dense_mm(li, w_d, kt_n, rhs, n_late, w_bufs=3):
                wmap = {}

                def ready(mt, ps, partial=False):
                    w_sb = load_w(li, w_d, kt_n, mt, BF16, bufs=w_bufs)
                    wmap[mt] = w_sb
                    for kt in range(kt_n - n_late):
                        for c in range(nch):
                            nc.tensor.matmul(ps[c][:], w_sb[:, kt, :],
                                             rhs[:, kt, ts(c, NCHUNK)],
                                             start=(kt == 0),
                                             stop=(partial and
                                                   kt == kt_n - n_late - 1))

                def tail(mt, ps, fresh=False):
                    w_sb = wmap.pop(mt)
                    for kt in range(kt_n - n_late, kt_n):
                        for c in range(nch):
                            nc.tensor.matmul(ps[c][:], w_sb[:, kt, :],
                                             rhs[:, kt, ts(c, NCHUNK)],
                                             start=(fresh and
                                                    kt == kt_n - n_late),
                                             stop=(kt == kt_n - 1))
                return ready, tail

            H1 = sb.tile([P, MT2, bc], BF16, tag="h1q", name="H1")
            r1, t1 = dense_mm(1, d["w1t"], MT1, H0, 6, w_bufs=6)
            fin1 = emit_layer(1, MT2, H1, r1, t1, True, fin0,
                              split=13, tail_s=3)

            H2 = sb.tile([P, MT3, bc], BF16, tag="h0q", name="H2")
            r2, t2 = dense_mm(2, d["w2t"], MT2, H1, 3, w_bufs=6)
            fin2 = emit_layer(2, MT3, H2, r2, t2, True, fin1,
                              split=6, tail_s=2)

            # ---- output layer: out[1, bc] = act3 @ Wout.T + bout ----
            N_LATE3 = 2
            ps3 = [pspool.tile([P, NCHUNK], F32, tag="ps", name="ps3")
                   for _ in range(nch)]
            for kt in range(KT3 - N_LATE3):
                for c in range(nch):
                    nc.tensor.matmul(ps3[c][0:1, :], w3_sb[:, kt:kt + 1],
                                     H2[:, kt, ts(c, NCHUNK)],
                                     start=(kt == 0), stop=False)
            fin2()
            for kt in range(KT3 - N_LATE3, KT3):
                for c in range(nch):
                    nc.tensor.matmul(ps3[c][0:1, :], w3_sb[:, kt:kt + 1],
                                     H2[:, kt, ts(c, NCHUNK)],
                                     start=False, stop=(kt == KT3 - 1))
            for c in range(nch):
                oc = wpool.tile([1, NCHUNK], F32, tag="oc", name="oc", bufs=4)
                nc.scalar.activation(oc[:], ps3[c][0:1, :],
                                     AF.Identity, bias=bout_sb[:])
                nc.sync.dma_start(out_d.ap()[:, ts(c, NCHUNK)], oc[:])

    nc.compile()
    return nc


# ---------------------------------------------------------------------------
# host-side prep + execution
# ---------------------------------------------------------------------------

_NC_CACHE = {}


def _get_nc(bc=BC):
    if bc not in _NC_CACHE:
        _NC_CACHE[bc] = build_nc(bc)
    return _NC_CACHE[bc]


def _bf16(a):
    return np.ascontiguousarray(a).astype(ml_dtypes.bfloat16)


def prep_in_maps(inputs, bc=BC, n_cores=N_CORES):
    xv = np.asarray(inputs["xv"], dtype=np.float32)
    W0 = np.asarray(inputs["W0"], dtype=np.float32)
    W1 = np.asarray(inputs["W1"], dtype=np.float32)
    W2 = np.asarray(inputs["W2"], dtype=np.float32)
    Wout = np.asarray(inputs["Wout"], dtype=np.float32)
    bout = np.asarray(inputs["bout"], dtype=np.float32)

    pii = np.zeros((FIELD, NPAIR_PAD), np.float32)
    pjj = np.zeros((FIELD, NPAIR_PAD), np.float32)
    pii[_II, np.arange(NPAIR)] = 1.0
    pjj[_JJ, np.arange(NPAIR)] = 1.0

    def strip_tile(wt, kt_n, mt_n):
        # wt: [K, M] -> [mt, p, kt, j] = wt[kt*128 + p, mt*128 + j]
        return np.ascontiguousarray(
            wt.reshape(kt_n, P, mt_n, P).transpose(2, 1, 0, 3))

    # xv part of W0 (first 256 k): bf16; qi part: /255 folded in, fp8 e5m2
    w0q = np.zeros((NPAIR_PAD, D1), np.float32)
    w0q[:NPAIR, :] = W0[:, F:].T / 255.0
    shared = {
        "pii": _bf16(pii),
        "pjj": _bf16(pjj),
        "w0x": _bf16(strip_tile(W0[:, :F].T, 2, MT1)),
        "w0q": strip_tile(w0q, QKT, MT1).astype(ml_dtypes.float8_e5m2),
        "w1t": _bf16(strip_tile(W1.T, MT1, MT2)),
        "w2t": _bf16(strip_tile(W2.T, MT2, MT3)),
        "w3t": _bf16(Wout.reshape(KT3, P).T),
        "bout3": bout.reshape(1, 1).astype(np.float32),
    }
    for li, (g, be, mt_n) in enumerate([
            (inputs["g0"], inputs["be0"], MT1),
            (inputs["g1"], inputs["be1"], MT2),
            (inputs["g2"], inputs["be2"], MT3)]):
        g = np.asarray(g, np.float32).reshape(mt_n, P).T
        be = np.asarray(be, np.float32).reshape(mt_n, P).T
        shared[f"gbe{li}"] = np.ascontiguousarray(
            np.stack([g, be], axis=-1), dtype=np.float32)

    in_maps = []
    for c in range(n_cores):
        xs = xv[c * bc:(c + 1) * bc, :]                     # [bc, F]
        xt = _bf16(xs.T.reshape(2, P, bc))
        m = dict(shared)
        m["xt"] = xt
        in_maps.append(m)
    return in_maps


def kernel(**inputs):
    nc = _get_nc(BC)
    in_maps = prep_in_maps(inputs)
    res = run_bass_kernel_spmd(nc, in_maps, core_ids=list(range(N_CORES)))
    out = np.concatenate(
        [res.results[c]["out"].reshape(BC) for c in range(N_CORES)])
    return out.reshape(B, 1).astype(np.float32)


# revision 12
# speedup vs baseline: 4.1875x; 1.7667x over previous
"""DeepQI dense MLP on 8 Trainium2 NeuronCores.

Strategy (data-parallel, feature-major):
  - Shard batch B=16384 across 8 cores (2048 rows each); weights replicated.
  - Feature-major on-chip layout: activations are [128, KT, B_c] SBUF tiles
    (feature on partitions, batch on free dim) so every layer is a matmul
    with K on partitions and BatchNorm stats are free-dim reductions.
  - Pairwise features qi.T = x_i * x_j are built on-chip (selection matmuls
    PII/PJJ run concurrently as 64-row PE tiles), stored as fp8e4m3; the
    reference's /255 is folded into W0's qi columns, which are stored as
    fp8e5m2 (/255 keeps them in e5m2's normal range).  The 16 qi k-tiles of
    layer 0 then run as 8 fp8 DoubleRow matmuls (2x PE throughput).  The qi
    path carries ~0.01% of h0's variance (the /255 damping), so fp8 error
    is negligible end-to-end.
  - BatchNorm is sync-BN via two small AllReduces per layer, split
    asymmetrically (most strips in AR1, 2-3 tail strips in AR2) so AR1 hides
    under this layer's tail matmuls and AR2's chain hides under the NEXT
    layer's matmuls: the next layer starts on the k-tiles whose relu is
    already done (2 strips interleaved in PSUM = ~24 us of cover), deferring
    the late k-tiles to the end of each accumulation.  bn_stats run directly
    on PSUM so they never wait for the ACT copy.  Linear biases cancel in BN
    and are dropped.
  - Matmuls in bf16 (fp32 PSUM accumulate) except the qi DoubleRow part.
"""

import numpy as np
import ml_dtypes

import concourse.bass as bass
import concourse.mybir as mybir
import concourse.tile as tile
from concourse import bacc
from concourse.bass import ts
from concourse.bass_utils import run_bass_kernel_spmd

N_CORES = 8
P = 128
B = 16384
BC = B // N_CORES  # 2048 batch rows per core
F = 256            # xv feature dim
FIELD = 64
NPAIR = 2016
NPAIR_PAD = 2048   # pad pairs to 16 full k-tiles
D1, D2, D3 = 2048, 2048, 1024
NCHUNK = 512       # matmul moving free dim (one PSUM bank)
EPS = 1e-5

BF16 = mybir.dt.bfloat16
F32 = mybir.dt.float32
F8E4 = mybir.dt.float8e4   # e4m3
F8E5 = mybir.dt.float8e5   # e5m2
AF = mybir.ActivationFunctionType
ALU = mybir.AluOpType
DR = mybir.MatmulPerfMode.DoubleRow

_II, _JJ = np.triu_indices(FIELD, k=1)

KT0 = (F + NPAIR_PAD) // P          # 18 (2 bf16 + 16 fp8)
QKT = NPAIR_PAD // P                 # 16 qi k-tiles -> 8 DoubleRow pairs
MT1, MT2, MT3 = D1 // P, D2 // P, D3 // P   # 16, 16, 8
KT3 = D3 // P                        # 8

# (name, shape, mybir dtype) of every kernel input -- used by test.py's
# null kernel to mirror the I/O signature.
INPUT_SPECS = [
    ("xt", [2, P, BC], BF16),
    ("pii", [FIELD, NPAIR_PAD], BF16),
    ("pjj", [FIELD, NPAIR_PAD], BF16),
    ("w0x", [MT1, P, 2, P], BF16),
    ("w0q", [MT1, P, QKT, P], F8E5),
    ("w1t", [MT2, P, MT1, P], BF16),
    ("w2t", [MT3, P, MT2, P], BF16),
    ("w3t", [P, KT3], BF16),
    ("gbe0", [P, MT1, 2], F32),
    ("gbe1", [P, MT2, 2], F32),
    ("gbe2", [P, MT3, 2], F32),
    ("bout3", [1, 1], F32),
]


def build_nc(bc=BC):
    assert bc % NCHUNK == 0
    nch = bc // NCHUNK

    nc = bacc.Bacc("TRN2", target_bir_lowering=False, debug=False,
                   num_devices=N_CORES)

    d = {name: nc.dram_tensor(name, shape, dt, kind="ExternalInput")
         for name, shape, dt in INPUT_SPECS}
    out_d = nc.dram_tensor("out", [1, bc], F32, kind="ExternalOutput")

    with tile.TileContext(nc) as tc:
        with (
            tc.tile_pool(name="sb", bufs=1) as sb,
            tc.tile_pool(name="wpool", bufs=4) as wpool,
            tc.tile_pool(name="pspool", bufs=8, space="PSUM") as pspool,
            tc.tile_pool(name="small", bufs=1) as small,
            tc.tile_pool(name="dram", bufs=1, space="DRAM") as dram,
        ):
            # ---- persistent activations ----
            X = sb.tile([P, 2, bc], BF16, tag="X", name="X")
            # Q shares H1's 64KB region (Q is read only during L0; H1 is
            # written from L1 on).  H2 likewise shares H0's region.
            Q = sb.tile([P, QKT, bc], F8E4, tag="h1q", name="Q")
            H0 = sb.tile([P, MT1, bc], BF16, tag="h0q", name="H0")

            nc.sync.dma_start(X[:], d["xt"].ap().rearrange("t p b -> p t b"))
            # PII on partitions 0-63, PJJ on 64-127: the two K=64 selection
            # matmuls then run CONCURRENTLY as 64x128 row tiles.
            pp_sb = small.tile([P, NPAIR_PAD], BF16, tag="pp_sb", name="pp_sb")
            nc.sync.dma_start(pp_sb[0:FIELD, :], d["pii"].ap())
            nc.sync.dma_start(pp_sb[FIELD:P, :], d["pjj"].ap())
            # duplicate of x features 0-63 on partitions 64-127 (the second
            # matmul's rhs must stream from SBUF partitions 64-127)
            xx_sb = small.tile([P, bc], BF16, tag="xx_sb", name="xx_sb")
            nc.sync.dma_start(xx_sb[FIELD:P, :], d["xt"].ap()[0, 0:FIELD, :])

            gbe_sb = {}
            for li, mt_n in [(0, MT1), (1, MT2), (2, MT3)]:
                g = small.tile([P, mt_n, 2], F32, tag=f"gbe{li}", name=f"gbe{li}")
                nc.sync.dma_start(g[:], d[f"gbe{li}"].ap())
                gbe_sb[li] = g
            w3_sb = small.tile([P, KT3], BF16, tag="w3_sb", name="w3_sb")
            nc.sync.dma_start(w3_sb[:], d["w3t"].ap())
            bout_sb = small.tile([1, 1], F32, tag="bout_sb", name="bout_sb")
            nc.sync.dma_start(bout_sb[:], d["bout3"].ap())
            eps_sb = small.tile([P, 1], F32, tag="eps_sb", name="eps_sb")
            nc.vector.memset(eps_sb[:], EPS)

            # ---- build qi tiles: Q[:, q, c] = (PII@x) * (PJJ@x), fp8e4 ----
            # chunk-outer: chunk c of every pair block is done after ~1/4 of
            # the DVE stream, so L0's first strip (also chunk-outer) starts
            # ~3/4 of the build earlier.
            for c in range(nch):
                for q in range(QKT):
                    a_ps = pspool.tile([P, NCHUNK], F32, tag="ps", name="aps")
                    b_ps = pspool.tile([P, NCHUNK], F32, tag="ps", name="bps")
                    nc.tensor.matmul(a_ps[:], pp_sb[0:FIELD, ts(q, P)],
                                     X[0:FIELD, 0, ts(c, NCHUNK)],
                                     start=True, stop=True,
                                     tile_position=(0, 0))
                    nc.tensor.matmul(b_ps[:], pp_sb[FIELD:P, ts(q, P)],
                                     xx_sb[FIELD:P, ts(c, NCHUNK)],
                                     start=True, stop=True,
                                     tile_position=(64, 0))
                    b_sb = wpool.tile([P, NCHUNK], BF16, tag="qtmp",
                                      name="b_sb", bufs=3)
                    nc.scalar.copy(b_sb[:], b_ps[:])
                    nc.vector.scalar_tensor_tensor(
                        out=Q[:, q, ts(c, NCHUNK)],
                        in0=a_ps[:], scalar=1.0, in1=b_sb[:],
                        op0=ALU.mult, op1=ALU.mult)

            # ------------------------------------------------------------------
            # layer machinery
            # ------------------------------------------------------------------
            def issue_ar(li, tag, mv, h0, h1):
                """Pack (mean/8, E[x^2]/8) for strips [h0,h1) and AllReduce."""
                HALF = h1 - h0
                hs = f"{li}_{tag}"
                mvh = mv[:, h0:h1, :]
                arp = small.tile([P, HALF, 2], F32, tag=f"arp_{hs}", name=f"arp_{hs}")
                nc.vector.tensor_scalar_mul(arp[:, :, 0], mvh[:, :, 0],
                                            1.0 / N_CORES)
                e2 = small.tile([P, HALF], F32, tag=f"e2_{hs}", name=f"e2_{hs}")
                nc.vector.tensor_mul(e2[:], mvh[:, :, 0], mvh[:, :, 0])
                nc.vector.tensor_add(e2[:], e2[:], mvh[:, :, 1])
                nc.vector.tensor_scalar_mul(arp[:, :, 1], e2[:], 1.0 / N_CORES)
                arin = dram.tile([P, HALF, 2], F32, tag=f"arin_{hs}", name=f"arin_{hs}")
                arout = dram.tile([P, HALF, 2], F32, tag=f"arout_{hs}", name=f"arout_{hs}")
                nc.sync.dma_start(arin[:], arp[:])
                nc.gpsimd.collective_compute(
                    "AllReduce", ALU.add,
                    replica_groups=[list(range(N_CORES))],
                    ins=[arin.opt()], outs=[arout.opt()])
                gl = small.tile([P, HALF, 2], F32, tag=f"gl_{hs}", name=f"gl_{hs}")
                nc.sync.dma_start(gl[:], arout[:])
                return gl

            def st_from_gl(li, tag, gl, h0, h1):
                """s = g / sqrt(var+eps); t = be - mean*s for strips [h0,h1)."""
                HALF = h1 - h0
                hs = f"{li}_{tag}"
                var = small.tile([P, HALF], F32, tag=f"var_{hs}", name=f"var_{hs}")
                nc.vector.tensor_mul(var[:], gl[:, :, 0], gl[:, :, 0])
                nc.vector.tensor_sub(var[:], gl[:, :, 1], var[:])
                sd = small.tile([P, HALF], F32, tag=f"sd_{hs}", name=f"sd_{hs}")
                nc.scalar.activation(sd[:], var[:], AF.Sqrt, bias=eps_sb[:])
                s_t = small.tile([P, HALF], F32, tag=f"s_{hs}", name=f"s_{hs}")
                nc.vector.reciprocal(s_t[:], sd[:])
                nc.vector.tensor_mul(s_t[:], s_t[:], gbe_sb[li][:, h0:h1, 0])
                t_t = small.tile([P, HALF], F32, tag=f"t_{hs}", name=f"t_{hs}")
                nc.vector.tensor_mul(t_t[:], gl[:, :, 0], s_t[:])
                nc.vector.tensor_sub(t_t[:], gbe_sb[li][:, h0:h1, 1], t_t[:])
                return s_t, t_t

            def relu_strip(h_buf, mt, s_t, t_t, j, on_act):
                s_ap = s_t[:, j:j + 1]
                t_ap = t_t[:, j:j + 1]
                if on_act:
                    nc.scalar.activation(h_buf[:, mt, :], h_buf[:, mt, :],
                                         AF.Relu, bias=t_ap, scale=s_ap)
                else:
                    nc.vector.tensor_scalar(
                        out=h_buf[:, mt, :], in0=h_buf[:, mt, :],
                        scalar1=s_ap, scalar2=t_ap,
                        op0=ALU.mult, op1=ALU.add)
                    nc.vector.tensor_scalar_max(
                        h_buf[:, mt, :], h_buf[:, mt, :], 0.0)

            def load_w(li, w_d, kt_n, mt, wdt, bufs=3):
                w_sb = wpool.tile([P, kt_n, P], wdt, tag=f"w_{kt_n}_{wdt}",
                                  name=f"w{li}_sb", bufs=bufs)
                nc.sync.dma_start(w_sb[:], w_d.ap()[mt])
                return w_sb

            def copy_out(h_buf, mt, ps, on_act):
                # GPSIMD cannot read PSUM, so these all go on ACT
                for c in range(nch):
                    nc.scalar.copy(h_buf[:, mt, ts(c, NCHUNK)], ps[c][:])

            def stats_psum(li, mt, ps, stats6, mv):
                for c in range(nch):
                    nc.vector.bn_stats(stats6[:, mt, c, :], ps[c][:])
                nc.vector.bn_aggr(mv[:, mt, :], stats6[:, mt, :, :])

            def stats_sbuf(li, h_buf, mt, stats6, mv):
                for c in range(nch):
                    nc.vector.bn_stats(stats6[:, mt, c, :],
                                       h_buf[:, mt, ts(c, NCHUNK)])
                nc.vector.bn_aggr(mv[:, mt, :], stats6[:, mt, :, :])

            N_SPLIT = 4   # strips per layer that drain ready-k partials so
                          # the prev layer's tail AR hides under deep PE work

            def emit_layer(li, mt_n, h_buf, mm_ready, mm_tail, has_late,
                           prev_finisher, split, tail_s):
                stats6 = small.tile([P, 16, nch, 6], F32,
                                    tag="st6", name=f"st6_{li}")
                mv = small.tile([P, mt_n, 2], F32, tag=f"mv_{li}",
                                name=f"mv_{li}")

                def alloc_ps():
                    return [pspool.tile([P, NCHUNK], F32, tag="ps", name="mps")
                            for _ in range(nch)]

                if has_late:
                    # phase 1: ready-k partial sums of the first N_SPLIT
                    # strips, drained to SBUF (frees PSUM -> deep cover)
                    for s in range(N_SPLIT):
                        ps = alloc_ps()
                        mm_ready(s, ps, partial=True)
                        copy_out(h_buf, s, ps, on_act=(s % 2 == 0))
                    prev_finisher()
                    # phase 2: late-k contributions, added in place
                    for s in range(N_SPLIT):
                        ps = alloc_ps()
                        mm_tail(s, ps, fresh=True)
                        for c in range(nch):
                            nc.vector.tensor_add(
                                h_buf[:, s, ts(c, NCHUNK)],
                                h_buf[:, s, ts(c, NCHUNK)], ps[c][:])
                        stats_sbuf(li, h_buf, s, stats6, mv)
                    start_s = N_SPLIT
                else:
                    if prev_finisher is not None:
                        prev_finisher()
                    start_s = 0

                gl1 = None
                for mt in range(start_s, mt_n):
                    ps = alloc_ps()
                    mm_ready(mt, ps, partial=False)
                    mm_tail(mt, ps, fresh=False)
                    stats_psum(li, mt, ps, stats6, mv)
                    copy_out(h_buf, mt, ps, on_act=(mt % 2 == 0))
                    if mt == split - 1:
                        gl1 = issue_ar(li, "a", mv, 0, split)

                gl2 = issue_ar(li, "b", mv, split, mt_n)
                st1 = st_from_gl(li, "a", gl1, 0, split)
                for j in range(split):
                    relu_strip(h_buf, j, st1[0], st1[1], j, on_act=(j % 2 == 0))

                def finisher():
                    st2 = st_from_gl(li, "b", gl2, split, mt_n)
                    for j in range(mt_n - split):
                        relu_strip(h_buf, split + j, st2[0], st2[1], j,
                                   on_act=False)
                return finisher

            # ---- layer 0: X (2 bf16 kts) + Q (8 fp8 DoubleRow pairs) ----
            def l0_mm(mt, ps, partial=False):
                wx = load_w(0, d["w0x"], 2, mt, BF16)
                wq = load_w("0q", d["w0q"], QKT, mt, F8E5)
                for c in range(nch):
                    for kt in range(2):
                        nc.tensor.matmul(ps[c][:], wx[:, kt, :],
                                         X[:, kt, ts(c, NCHUNK)],
                                         start=(kt == 0), stop=False)
                    for dkt in range(QKT // 2):
                        nc.tensor.matmul(
                            ps[c][:], wq[:, 2 * dkt:2 * dkt + 2, :],
                            Q[:, 2 * dkt:2 * dkt + 2, ts(c, NCHUNK)],
                            start=False, stop=(dkt == QKT // 2 - 1),
                            perf_mode=DR)

            fin0 = emit_layer(0, MT1, H0, l0_mm,
                              lambda mt, ps, fresh=False: None,
                              False, None, split=10, tail_s=6)

            # ---- layers 1, 2: dense bf16 over previous h ----
            def dense_mm(li, w_d, kt_n, rhs, n_late, w_bufs=3):
                wmap = {}

                def ready(mt, ps, partial=False):
                    w_sb = load_w(li, w_d, kt_n, mt, BF16, bufs=w_bufs)
                    wmap[mt] = w_sb
                    for kt in range(kt_n - n_late):
                        for c in range(nch):
                            nc.tensor.matmul(ps[c][:], w_sb[:, kt, :],
                                             rhs[:, kt, ts(c, NCHUNK)],
                                             start=(kt == 0),
                                             stop=(partial and
                                                   kt == kt_n - n_late - 1))

                def tail(mt, ps, fresh=False):
                    w_sb = wmap.pop(mt)
                    for kt in range(kt_n - n_late, kt_n):
                        for c in range(nch):
                            nc.tensor.matmul(ps[c][:], w_sb[:, kt, :],
                                             rhs[:, kt, ts(c, NCHUNK)],
                                             start=(fresh and
                                                    kt == kt_n - n_late),
                                             stop=(kt == kt_n - 1))
                return ready, tail

            H1 = sb.tile([P, MT2, bc], BF16, tag="h1q", name="H1")
            r1, t1 = dense_mm(1, d["w1t"], MT1, H0, 6, w_bufs=6)
            fin1 = emit_layer(1, MT2, H1, r1, t1, True, fin0,
                              split=13, tail_s=3)

            H2 = sb.tile([P, MT3, bc], BF16, tag="h0q", name="H2")
            r2, t2 = dense_mm(2, d["w2t"], MT2, H1, 3, w_bufs=6)
            fin2 = emit_layer(2, MT3, H2, r2, t2, True, fin1,
                              split=6, tail_s=2)

            # ---- output layer: out[1, bc] = act3 @ Wout.T + bout ----
            N_LATE3 = 2
            ps3 = [pspool.tile([P, NCHUNK], F32, tag="ps", name="ps3")
                   for _ in range(nch)]
            for kt in range(KT3 - N_LATE3):
                for c in range(nch):
                    nc.tensor.matmul(ps3[c][0:1, :], w3_sb[:, kt:kt + 1],
                                     H2[:, kt, ts(c, NCHUNK)],
                                     start=(kt == 0), stop=False)
            fin2()
            for kt in range(KT3 - N_LATE3, KT3):
                for c in range(nch):
                    nc.tensor.matmul(ps3[c][0:1, :], w3_sb[:, kt:kt + 1],
                                     H2[:, kt, ts(c, NCHUNK)],
                                     start=False, stop=(kt == KT3 - 1))
            for c in range(nch):
                oc = wpool.tile([1, NCHUNK], F32, tag="oc", name="oc", bufs=4)
                nc.scalar.activation(oc[:], ps3[c][0:1, :],
                                     AF.Identity, bias=bout_sb[:])
                nc.sync.dma_start(out_d.ap()[:, ts(c, NCHUNK)], oc[:])

    nc.compile()
    return nc


# ---------------------------------------------------------------------------
# host-side prep + execution
# ---------------------------------------------------------------------------

_NC_CACHE = {}


def _get_nc(bc=BC):
    if bc not in _NC_CACHE:
        _NC_CACHE[bc] = build_nc(bc)
    return _NC_CACHE[bc]


def _bf16(a):
    return np.ascontiguousarray(a).astype(ml_dtypes.bfloat16)


def prep_in_maps(inputs, bc=BC, n_cores=N_CORES):
    xv = np.asarray(inputs["xv"], dtype=np.float32)
    W0 = np.asarray(inputs["W0"], dtype=np.float32)
    W1 = np.asarray(inputs["W1"], dtype=np.float32)
    W2 = np.asarray(inputs["W2"], dtype=np.float32)
    Wout = np.asarray(inputs["Wout"], dtype=np.float32)
    bout = np.asarray(inputs["bout"], dtype=np.float32)

    pii = np.zeros((FIELD, NPAIR_PAD), np.float32)
    pjj = np.zeros((FIELD, NPAIR_PAD), np.float32)
    pii[_II, np.arange(NPAIR)] = 1.0
    pjj[_JJ, np.arange(NPAIR)] = 1.0

    def strip_tile(wt, kt_n, mt_n):
        # wt: [K, M] -> [mt, p, kt, j] = wt[kt*128 + p, mt*128 + j]
        return np.ascontiguousarray(
            wt.reshape(kt_n, P, mt_n, P).transpose(2, 1, 0, 3))

    # xv part of W0 (first 256 k): bf16; qi part: /255 folded in, fp8 e5m2
    w0q = np.zeros((NPAIR_PAD, D1), np.float32)
    w0q[:NPAIR, :] = W0[:, F:].T / 255.0
    shared = {
        "pii": _bf16(pii),
        "pjj": _bf16(pjj),
        "w0x": _bf16(strip_tile(W0[:, :F].T, 2, MT1)),
        "w0q": strip_tile(w0q, QKT, MT1).astype(ml_dtypes.float8_e5m2),
        "w1t": _bf16(strip_tile(W1.T, MT1, MT2)),
        "w2t": _bf16(strip_tile(W2.T, MT2, MT3)),
        "w3t": _bf16(Wout.reshape(KT3, P).T),
        "bout3": bout.reshape(1, 1).astype(np.float32),
    }
    for li, (g, be, mt_n) in enumerate([
            (inputs["g0"], inputs["be0"], MT1),
            (inputs["g1"], inputs["be1"], MT2),
            (inputs["g2"], inputs["be2"], MT3)]):
        g = np.asarray(g, np.float32).reshape(mt_n, P).T
        be = np.asarray(be, np.float32).reshape(mt_n, P).T
        shared[f"gbe{li}"] = np.ascontiguousarray(
            np.stack([g, be], axis=-1), dtype=np.float32)

    in_maps = []
    for c in range(n_cores):
        xs = xv[c * bc:(c + 1) * bc, :]                     # [bc, F]
        xt = _bf16(xs.T.reshape(2, P, bc))
        m = dict(shared)
        m["xt"] = xt
        in_maps.append(m)
    return in_maps


def kernel(**inputs):
    nc = _get_nc(BC)
    in_maps = prep_in_maps(inputs)
    res = run_bass_kernel_spmd(nc, in_maps, core_ids=list(range(N_CORES)))
    out = np.concatenate(
        [res.results[c]["out"].reshape(BC) for c in range(N_CORES)])
    return out.reshape(B, 1).astype(np.float32)
